# revision 1
# baseline (speedup 1.0000x reference)
"""CRF layer gradient kernel for 8 TRN2 NeuronCores.

Strategy: data-parallel over the N=2048 words axis (256 words/core).
The forward-backward DP is done in the exp domain (scaled forward-backward):
with ETs = exp(T)/c for a fixed scale constant c, the recurrences
  A[i+1] = (A[i] * E[i]) @ ETs          (A[0] = 1)
  B[i-1] = ETs.T @ (B[i] * E[i])        (B[63] = 1/z, z = sum_k A[63]*E[63])
give per-position marginals with a SINGLE per-word normalizer folded into B:
  p1[i]  = A[i]*B[i]*E[i]
  p2[i] ~= (A[i]E[i]) (x) ETs (B[i+1]E[i+1])   (constant c absorbed on host)
This removes all logsumexp/softmax-max machinery: the scan is one elementwise
multiply + one [32x32]-block matmul per step, batched over words.

Device layout: "packed" [128 = 4 chains x 32 labels, 64 words x 64 positions].
Core outputs: dw partial [32,512] (on-device G.T @ data matmul), plus AE/BEn
marginal factors, from which the host forms the tiny dT matrix.
"""

import os
import sys

import numpy as np

sys.path.insert(0, "/opt/trn_rl_repo")

import concourse.bass as bass
import concourse.tile as tile
from concourse import bacc, mybir
from concourse.bass_utils import run_bass_kernel_spmd

N, M, K, D = 2048, 64, 32, 512
NC = 8
WPC = N // NC          # 256 words per core
RPC = WPC * M          # 16384 rows per core
CHAT = 60.0            # scan scale constant (typical per-step growth)
F32 = mybir.dt.float32
BF16 = mybir.dt.bfloat16

_CACHE = {}


def _build_module():
    nc = bacc.Bacc("TRN2", target_bir_lowering=False, debug=False)

    # --- DRAM I/O ---
    dt_d = nc.dram_tensor("dt", [D, RPC], BF16, kind="ExternalInput")       # data.T
    dn_d = nc.dram_tensor("dn", [RPC, D], BF16, kind="ExternalInput")       # data natural
    wt_d = nc.dram_tensor("wt", [128, 4, K], BF16, kind="ExternalInput")    # W.T packed
    etf_d = nc.dram_tensor("etf", [128, 128], F32, kind="ExternalInput")    # diag4(exp(T)/c)
    etb_d = nc.dram_tensor("etb", [128, 128], F32, kind="ExternalInput")    # diag4((exp(T)/c).T)
    oz_d = nc.dram_tensor("oz", [128, 4], F32, kind="ExternalInput")        # block ones
    ob_d = nc.dram_tensor("ob", [4, 128], F32, kind="ExternalInput")        # block ones T
    id_d = nc.dram_tensor("id32", [128, K], F32, kind="ExternalInput")      # stacked identity
    oh_d = nc.dram_tensor("oh", [128, 4096], F32, kind="ExternalInput")     # onehot_T packed
    dw_d = nc.dram_tensor("dw", [K, D], F32, kind="ExternalOutput")
    ae_d = nc.dram_tensor("ae", [128, 4096], F32, kind="ExternalOutput")
    be_d = nc.dram_tensor("be", [128, 4096], F32, kind="ExternalOutput")

    with tile.TileContext(nc) as tc:
        _kernel_body(tc, nc, dt_d, dn_d, wt_d, etf_d, etb_d, oz_d, ob_d,
                     id_d, oh_d, dw_d, ae_d, be_d)
    nc.compile()
    return nc


def _kernel_body(tc, nc, dt_d, dn_d, wt_d, etf_d, etb_d, oz_d, ob_d,
                 id_d, oh_d, dw_d, ae_d, be_d):
    from contextlib import ExitStack
    ctx = ExitStack()
    with ctx:
        consts = ctx.enter_context(tc.tile_pool(name="consts", bufs=1))
        big = ctx.enter_context(tc.tile_pool(name="big", bufs=1))
        dnp = ctx.enter_context(tc.tile_pool(name="dnp", bufs=64))

        wt_t = consts.tile([128, 4, K], BF16)
        nc.sync.dma_start(wt_t[:], wt_d.ap())
        etf_t = consts.tile([128, 128], F32)
        nc.sync.dma_start(etf_t[:], etf_d.ap())
        etb_t = consts.tile([128, 128], F32)
        nc.sync.dma_start(etb_t[:], etb_d.ap())
        oz_t = consts.tile([128, 4], F32)
        nc.sync.dma_start(oz_t[:], oz_d.ap())
        ob_t = consts.tile([4, 128], F32)
        nc.sync.dma_start(ob_t[:], ob_d.ap())
        id_t = consts.tile([128, K], F32)
        nc.sync.dma_start(id_t[:], id_d.ap())
        oh_t = big.tile([128, 4096], F32, tag="oh")
        nc.sync.dma_start(oh_t[:], oh_d.ap())

        e_t = big.tile([128, 4096], F32, tag="e")
        einv_t = big.tile([128, 4096], F32, tag="einv")
        ae_t = big.tile([128, 4096], F32, tag="ae")
        be_t = big.tile([128, 4096], F32, tag="be")
        g_t = big.tile([128, 4096], F32, tag="g")     # p1 scratch
        dn_tiles = [None] * 128                        # natural data, streamed

        # ---- Phase A: dots + exp(+-dots), 2 chains per PSUM half-bank ----
        # (PE matmul base partitions are restricted to {0, 32, 64}, so a
        # bank stacks chains 2h, 2h+1 at offsets 0/32.)
        with tc.tile_pool(name="dotp", bufs=3, space="PSUM") as dotp, \
             tc.tile_pool(name="dtpool", bufs=8) as dtpool:
            for s8 in range(8):
                for h in range(2):
                    bank = dotp.tile([64, 512], F32)
                    for cc in range(2):
                        c = 2 * h + cc
                        t = 8 * c + s8
                        for g in range(4):
                            dtt = dtpool.tile([128, 512], BF16)
                            nc.sync.dma_start(
                                dtt[:], dt_d.ap()[128 * g:128 * g + 128,
                                                  512 * t:512 * t + 512])
                            nc.tensor.matmul(
                                bank[32 * cc:32 * cc + 32, :],
                                wt_t[:, g, :], dtt[:],
                                start=(g == 0), stop=(g == 3))
                    sl = slice(512 * s8, 512 * s8 + 512)
                    pr = slice(64 * h, 64 * h + 64)
                    nc.scalar.activation(e_t[pr, sl], bank[:],
                                         mybir.ActivationFunctionType.Exp)
                    nc.scalar.activation(einv_t[pr, sl], bank[:],
                                         mybir.ActivationFunctionType.Exp,
                                         scale=-1.0)

        def esl(i):      # strided [128, 64] slice of a packed big tile
            return slice(i, 4096, 64)

        # ---- Phase B: forward scan ----
        with tc.tile_pool(name="scanp", bufs=3, space="PSUM") as scanp, \
             tc.tile_pool(name="zp", bufs=2, space="PSUM") as zp, \
             tc.tile_pool(name="rzp", bufs=1) as rzp:
            nc.vector.tensor_copy(ae_t[:, esl(0)], e_t[:, esl(0)])  # AE[0]=E[0]
            acur = scanp.tile([128, 64], F32, tag="a")
            nc.tensor.matmul(acur[:], etf_t[:], ae_t[:, esl(0)],
                             start=True, stop=True)
            for i in range(1, 63):
                nc.vector.tensor_mul(ae_t[:, esl(i)], acur[:], e_t[:, esl(i)])
                anext = scanp.tile([128, 64], F32, tag="a")
                nc.tensor.matmul(anext[:], etf_t[:], ae_t[:, esl(i)],
                                 start=True, stop=True)
                acur = anext
            nc.vector.tensor_mul(ae_t[:, esl(63)], acur[:], e_t[:, esl(63)])

            # z per word, rz = 1/z broadcast to all 128 partitions
            z_ps = zp.tile([4, 64], F32, tag="z")
            nc.tensor.matmul(z_ps[:], oz_t[:], ae_t[:, esl(63)],
                             start=True, stop=True)
            rz_s = rzp.tile([4, 64], F32)
            nc.vector.reciprocal(rz_s[:], z_ps[:])
            rzb_ps = zp.tile([128, 64], F32, tag="rzb")
            nc.tensor.matmul(rzb_ps[:], ob_t[:], rz_s[:], start=True, stop=True)
            rz_t = rzp.tile([128, 64], F32)
            nc.vector.tensor_copy(rz_t[:], rzb_ps[:])

            # ---- natural-layout data loads (used by Phase E; queued now) ----
            for j in range(128):
                dn_tiles[j] = dnp.tile([128, 512], BF16, tag="dn", name=f"dn{j}")
                nc.sync.dma_start(dn_tiles[j][:],
                                  dn_d.ap()[128 * j:128 * j + 128, :])

            # ---- Phase C: backward scan (rz folded into B[63]) ----
            nc.vector.tensor_mul(be_t[:, esl(63)], rz_t[:], e_t[:, esl(63)])
            bcur = scanp.tile([128, 64], F32, tag="a")
            nc.tensor.matmul(bcur[:], etb_t[:], be_t[:, esl(63)],
                             start=True, stop=True)
            for i in range(62, 0, -1):
                nc.vector.tensor_mul(be_t[:, esl(i)], bcur[:], e_t[:, esl(i)])
                bnext = scanp.tile([128, 64], F32, tag="a")
                nc.tensor.matmul(bnext[:], etb_t[:], be_t[:, esl(i)],
                                 start=True, stop=True)
                bcur = bnext
            nc.vector.tensor_mul(be_t[:, esl(0)], bcur[:], e_t[:, esl(0)])

        # ---- Phase D: G = onehot - AE*BEn*Einv (packed, full width) ----
        nc.vector.tensor_mul(g_t[:], ae_t[:], be_t[:])
        nc.vector.tensor_mul(e_t[:], g_t[:], einv_t[:])      # e_t dead: = p1

        nc.vector.tensor_sub(g_t[:], oh_t[:], e_t[:])        # G = oh - p1
        # PE transpose lhsT base partition must be in {0,32,64}: chain 3
        # (base 96) needs a relocated copy.
        g3_t = big.tile([32, 4096], F32, tag="g3", name="g3_t")
        nc.vector.tensor_copy(g3_t[:], g_t[96:128, :])

        # ---- Phase E: per-chunk transpose of G + dw matmul ----
        with tc.tile_pool(name="trp", bufs=2, space="PSUM") as trp, \
             tc.tile_pool(name="dwp", bufs=1, space="PSUM") as dwp, \
             tc.tile_pool(name="gsb", bufs=2) as gsbp:
            dw_ps = dwp.tile([K, D], F32)
            for q in range(32):            # 4 chunks per iteration
                tr = trp.tile([128, 128], F32)
                for gg in range(4):
                    j = 4 * q + gg
                    c, jj = j // 32, j % 32
                    if c == 3:
                        src_ap = g3_t[:, 128 * jj:128 * jj + 128]
                        id_ap = id_t[0:32, :]
                    else:
                        src_ap = g_t[32 * c:32 * c + 32,
                                     128 * jj:128 * jj + 128]
                        id_ap = id_t[32 * c:32 * c + 32, :]
                    nc.tensor.transpose(
                        tr[:, 32 * gg:32 * gg + 32], src_ap, id_ap)
                gsb = gsbp.tile([128, 128], BF16)
                nc.vector.tensor_copy(gsb[:], tr[:])
                for gg in range(4):
                    j = 4 * q + gg
                    nc.tensor.matmul(dw_ps[:],
                                     gsb[:, 32 * gg:32 * gg + 32],
                                     dn_tiles[j][:],
                                     start=(j == 0), stop=(j == 127))
            dw_sb = gsbp.tile([K, D], F32, tag="dwout")
            nc.vector.tensor_copy(dw_sb[:], dw_ps[:])
            nc.sync.dma_start(dw_d.ap(), dw_sb[:])

        nc.sync.dma_start(ae_d.ap(), ae_t[:])
        nc.sync.dma_start(be_d.ap(), be_t[:])


def _pack_T(x_core):
    """[16384(=4096*4 rows)] -> packed [128, 4096] view helper (labels/marginals).

    packed[32c+k, f] corresponds to natural row 4096c+f, label k.
    """
    raise NotImplementedError


def kernel(W, T, data, labels):
    W = np.asarray(W, np.float32)
    T = np.asarray(T, np.float32)
    data = np.asarray(data, np.float32)
    labels = np.asarray(labels, np.int32)

    import ml_dtypes
    bf16 = ml_dtypes.bfloat16

    ET = np.exp(T).astype(np.float32)
    ETs = (ET / CHAT).astype(np.float32)
    etf = np.zeros((128, 128), np.float32)
    etb = np.zeros((128, 128), np.float32)
    for c in range(4):
        etf[32 * c:32 * c + 32, 32 * c:32 * c + 32] = ETs       # lhsT=ETs: A@ETs
        etb[32 * c:32 * c + 32, 32 * c:32 * c + 32] = ETs.T     # lhsT=ETs.T: ETs@BE
    oz = np.zeros((128, 4), np.float32)
    ob = np.zeros((4, 128), np.float32)
    for c in range(4):
        oz[32 * c:32 * c + 32, c] = 1.0
        ob[c, 32 * c:32 * c + 32] = 1.0
    id32 = np.tile(np.eye(K, dtype=np.float32), (4, 1))
    wt = np.zeros((128, 4, K), np.float32)
    for g in range(4):
        wt[:, g, :] = W.T[128 * g:128 * g + 128, :]

    nc = _CACHE.get("nc")
    if nc is None:
        nc = _build_module()
        _CACHE["nc"] = nc

    in_maps = []
    for core in range(NC):
        dcore = data[core * WPC:(core + 1) * WPC].reshape(RPC, D)
        lcore = labels[core * WPC:(core + 1) * WPC].reshape(RPC)
        oh = np.zeros((128, 4096), np.float32)
        rows = np.arange(RPC)
        cc, ff = rows // 4096, rows % 4096
        oh[32 * cc + lcore, ff] = 1.0
        in_maps.append({
            "dt": np.ascontiguousarray(dcore.T).astype(bf16),
            "dn": dcore.astype(bf16),
            "wt": wt.astype(bf16),
            "etf": etf, "etb": etb, "oz": oz, "ob": ob, "id32": id32,
            "oh": oh,
        })

    _CACHE["last_in_maps"] = in_maps
    res = run_bass_kernel_spmd(nc, in_maps, list(range(NC)))
    results = res.results

    dw_sum = np.zeros((K, D), np.float64)
    Mmat = np.zeros((K, K), np.float64)
    for core in range(NC):
        r = results[core]
        dw_sum += r["dw"].astype(np.float64)
        ae = r["ae"].astype(np.float32)   # [128, 4096] packed
        be = r["be"].astype(np.float32)
        # unpack to natural [RPC, K]
        ae_n = ae.reshape(4, K, 4096).transpose(0, 2, 1).reshape(RPC, K)
        be_n = be.reshape(4, K, 4096).transpose(0, 2, 1).reshape(RPC, K)
        aer = ae_n.reshape(WPC, M, K)[:, :M - 1].reshape(-1, K)
        ben = be_n.reshape(WPC, M, K)[:, 1:].reshape(-1, K)
        Mmat += aer.T.astype(np.float64) @ ben.astype(np.float64)

    counts = np.zeros((K, K), np.float64)
    np.add.at(counts, (labels[:, :-1].ravel(), labels[:, 1:].ravel()), 1.0)

    meandw = (dw_sum / N).astype(np.float32)
    meandT = ((counts - (ET.astype(np.float64) / CHAT) * Mmat) / N).astype(np.float32)
    return np.concatenate([meandw.ravel(), meandT.ravel()]).astype(np.float32)



# revision 4
# speedup vs baseline: 1.2778x; 1.2778x over previous
"""CRF layer gradient kernel for 8 TRN2 NeuronCores (v2).

Strategy: data-parallel over N=2048 words (256 words/core, 4 chains x 64).
Scaled forward-backward in the exp domain with ETs = exp(T)/c:
  AE[i+1] = (AE[i] @ ETs) * E[i+1]         (AE[0] = E[0])
  BE[i-1] = (ETs @ BE[i]) * E[i-1]         (BE[63] = E[63], UNNORMALIZED)
Both scans run CONCURRENTLY; the per-word normalizer z = sum_k AE[63,k] is
computed on device afterwards and folded into einv (ez = einv * 1/z bcast),
so p1 = AE*BE*ez. G = p1 - oh is transposed per 128-row block on the PE and
fed to a col-tiled dw matmul; dw is reduced on device to [32, 512].

Packing: [128 = 4 chains x 32 labels, 4096 = 64 positions x 64 words]
(POSITION-major free dim: scan slices are contiguous [128, 64] blocks).
Host post-processing: mean-dw sign flip, dT from ae/be outputs + counts.
"""

import sys

import numpy as np

sys.path.insert(0, "/opt/trn_rl_repo")

import concourse.bass as bass
import concourse.tile as tile
from concourse import bacc, mybir
from concourse.bass_utils import run_bass_kernel_spmd

N, M, K, D = 2048, 64, 32, 512
NC = 8
WPC = N // NC          # 256 words per core
RPC = WPC * M          # 16384 rows per core
CHAT = 60.0            # scan scale constant
F32 = mybir.dt.float32
BF16 = mybir.dt.bfloat16

_CACHE = {}


def _build_module():
    nc = bacc.Bacc("TRN2", target_bir_lowering=False, debug=False)

    dt_d = nc.dram_tensor("dt", [4, D, 4096], BF16, kind="ExternalInput")
    dn_d = nc.dram_tensor("dn", [RPC, D], BF16, kind="ExternalInput")
    wt4_d = nc.dram_tensor("wt4", [128, 4, 128], BF16, kind="ExternalInput")
    etf_d = nc.dram_tensor("etf", [128, 128], F32, kind="ExternalInput")
    etb_d = nc.dram_tensor("etb", [128, 128], F32, kind="ExternalInput")
    oz_d = nc.dram_tensor("oz", [128, 4], F32, kind="ExternalInput")
    ob_d = nc.dram_tensor("ob", [4, 128], F32, kind="ExternalInput")
    on32_d = nc.dram_tensor("on32", [128, K], F32, kind="ExternalInput")
    id128_d = nc.dram_tensor("id128", [128, 128], BF16, kind="ExternalInput")
    oh_d = nc.dram_tensor("oh", [128, 4096], BF16, kind="ExternalInput")
    dw_d = nc.dram_tensor("dw", [K, D], F32, kind="ExternalOutput")
    ae_d = nc.dram_tensor("ae", [128, 4096], F32, kind="ExternalOutput")
    be_d = nc.dram_tensor("be", [128, 4096], F32, kind="ExternalOutput")

    with tile.TileContext(nc) as tc:
        _kernel_body(tc, nc, dt_d, dn_d, wt4_d, etf_d, etb_d, oz_d, ob_d,
                     on32_d, id128_d, oh_d, dw_d, ae_d, be_d)
    nc.compile()
    return nc


def _kernel_body(tc, nc, dt_d, dn_d, wt4_d, etf_d, etb_d, oz_d, ob_d,
                 on32_d, id128_d, oh_d, dw_d, ae_d, be_d):
    from contextlib import ExitStack
    ctx = ExitStack()
    with ctx:
        consts = ctx.enter_context(tc.tile_pool(name="consts", bufs=1))
        big = ctx.enter_context(tc.tile_pool(name="big", bufs=1))
        dtp = ctx.enter_context(tc.tile_pool(name="dtp", bufs=6))
        dnp = ctx.enter_context(tc.tile_pool(name="dnp", bufs=64))
        scr = ctx.enter_context(tc.tile_pool(name="scr", bufs=3))
        gsbp = ctx.enter_context(tc.tile_pool(name="gsbp", bufs=3))

        wt4_t = consts.tile([128, 4, 128], BF16)
        nc.sync.dma_start(wt4_t[:], wt4_d.ap())
        etf_t = consts.tile([128, 128], F32)
        nc.sync.dma_start(etf_t[:], etf_d.ap())
        etb_t = consts.tile([128, 128], F32)
        nc.sync.dma_start(etb_t[:], etb_d.ap())
        oz_t = consts.tile([128, 4], F32)
        nc.sync.dma_start(oz_t[:], oz_d.ap())
        ob_t = consts.tile([4, 128], F32)
        nc.sync.dma_start(ob_t[:], ob_d.ap())
        on32_t = consts.tile([128, K], F32)
        nc.sync.dma_start(on32_t[:], on32_d.ap())
        id128_t = consts.tile([128, 128], BF16)
        nc.sync.dma_start(id128_t[:], id128_d.ap())
        oh_t = big.tile([128, 4096], BF16, tag="oh")
        nc.sync.dma_start(oh_t[:], oh_d.ap())

        e_t = big.tile([128, 4096], BF16, tag="e")
        einv_t = big.tile([128, 4096], BF16, tag="einv")
        ez_t = big.tile([128, 4096], BF16, tag="ez")
        ae_t = big.tile([128, 4096], F32, tag="ae")
        be_t = big.tile([128, 4096], F32, tag="be")
        rzb_t = consts.tile([128, 64], F32)
        rz_t = consts.tile([4, 64], F32)

        # ---- Phase A: dots (col-tiled, 4 chains per PSUM bank) + exp ----
        with tc.tile_pool(name="dotp", bufs=2, space="PSUM") as dotp:
            for q in range(8):
                P = dotp.tile([128, 512], F32)
                for g in range(4):
                    for c in range(4):
                        dtt = dtp.tile([128, 512], BF16)
                        nc.sync.dma_start(
                            dtt[:], dt_d.ap()[c, 128 * g:128 * g + 128,
                                              512 * q:512 * q + 512])
                        nc.tensor.matmul(
                            P[32 * c:32 * c + 32, :],
                            wt4_t[:, g, 32 * c:32 * c + 32], dtt[:],
                            start=(g == 0), stop=(g == 3),
                            tile_position=(0, 32 * c))
                sl = slice(512 * q, 512 * q + 512)
                nc.scalar.activation(e_t[:, sl], P[:],
                                     mybir.ActivationFunctionType.Exp)
                nc.scalar.activation(einv_t[:, sl], P[:],
                                     mybir.ActivationFunctionType.Exp,
                                     scale=-1.0)

        # ---- Phase B: concurrent forward & backward scans ----
        with tc.tile_pool(name="scanp", bufs=6, space="PSUM") as scanp, \
             tc.tile_pool(name="zp", bufs=2, space="PSUM") as zp:
            nc.vector.tensor_copy(ae_t[:, 0:64], e_t[:, 0:64])
            nc.vector.tensor_copy(be_t[:, 4032:4096], e_t[:, 4032:4096])
            af = scanp.tile([128, 64], F32, tag="s")
            nc.tensor.matmul(af[:], etf_t[:], ae_t[:, 0:64],
                             start=True, stop=True)
            bb = scanp.tile([128, 64], F32, tag="s")
            nc.tensor.matmul(bb[:], etb_t[:], be_t[:, 4032:4096],
                             start=True, stop=True)
            for s in range(1, 64):
                sf = slice(64 * s, 64 * s + 64)
                sb = slice(64 * (63 - s), 64 * (63 - s) + 64)
                nc.vector.tensor_mul(ae_t[:, sf], af[:], e_t[:, sf])
                nc.vector.tensor_mul(be_t[:, sb], bb[:], e_t[:, sb])
                if s < 63:
                    af = scanp.tile([128, 64], F32, tag="s")
                    nc.tensor.matmul(af[:], etf_t[:], ae_t[:, sf],
                                     start=True, stop=True)
                    bb = scanp.tile([128, 64], F32, tag="s")
                    nc.tensor.matmul(bb[:], etb_t[:], be_t[:, sb],
                                     start=True, stop=True)

            # outputs for host-side dT (on scalar-engine DMA queue)
            nc.scalar.dma_start(ae_d.ap(), ae_t[:])
            nc.scalar.dma_start(be_d.ap(), be_t[:])

            # ---- z per word; ez = einv * (1/z) broadcast ----
            z_ps = zp.tile([128, 64], F32, tag="z")
            nc.tensor.matmul(z_ps[0:4, :], oz_t[:], ae_t[:, 4032:4096],
                             start=True, stop=True)
            nc.vector.reciprocal(rz_t[:], z_ps[0:4, :])
            rzb_ps = zp.tile([128, 64], F32, tag="z")
            nc.tensor.matmul(rzb_ps[:], ob_t[:], rz_t[:],
                             start=True, stop=True)
            nc.vector.tensor_copy(rzb_t[:], rzb_ps[:])

        ez3 = ez_t[:].rearrange("p (i w) -> p i w", i=64)
        ei3 = einv_t[:].rearrange("p (i w) -> p i w", i=64)
        rz3 = rzb_t[:].unsqueeze(1)          # [128, 1, 64]
        rz3b, ei3b = bass.broadcast_tensor_aps(rz3, ei3)
        nc.vector.tensor_mul(ez3, ei3b, rz3b)

        # ---- Phase E: G = p1 - oh per 128-col block; transpose; dw ----
        with tc.tile_pool(name="trp", bufs=2, space="PSUM") as trp, \
             tc.tile_pool(name="dwp", bufs=1, space="PSUM") as dwp, \
             tc.tile_pool(name="drp", bufs=1, space="PSUM") as drp:
            dwacc = dwp.tile([128, 512], F32)
            for jj in range(32):
                sl = slice(128 * jj, 128 * jj + 128)
                p1c = scr.tile([128, 128], F32, tag="p1")
                nc.vector.tensor_mul(p1c[:], ae_t[:, sl], be_t[:, sl])
                p1b = scr.tile([128, 128], BF16, tag="p1b")
                nc.vector.tensor_mul(p1b[:], p1c[:], ez_t[:, sl])
                gc = scr.tile([128, 128], BF16, tag="g")
                nc.vector.tensor_sub(gc[:], p1b[:], oh_t[:, sl])
                tr = trp.tile([128, 128], BF16)
                nc.tensor.transpose(tr[:], gc[:], id128_t[:])
                gsb = gsbp.tile([128, 128], BF16)
                nc.vector.tensor_copy(gsb[:], tr[:])
                for c in range(4):
                    j = 32 * c + jj
                    dnt = dnp.tile([128, 512], BF16, tag="dn")
                    nc.sync.dma_start(dnt[:],
                                      dn_d.ap()[128 * j:128 * j + 128, :])
                    nc.tensor.matmul(dwacc[32 * c:32 * c + 32, :],
                                     gsb[:, 32 * c:32 * c + 32], dnt[:],
                                     start=(jj == 0), stop=(jj == 31),
                                     tile_position=(0, 32 * c))

            dwsb = gsbp.tile([128, 512], F32, tag="dwsb")
            nc.vector.tensor_copy(dwsb[:], dwacc[:])
            dwred = drp.tile([K, 512], F32)
            nc.tensor.matmul(dwred[:], on32_t[:], dwsb[:],
                             start=True, stop=True)
            dwout = gsbp.tile([K, 512], F32, tag="dwout")
            nc.vector.tensor_copy(dwout[:], dwred[:])
            nc.scalar.dma_start(dw_d.ap(), dwout[:])


def kernel(W, T, data, labels):
    W = np.asarray(W, np.float32)
    T = np.asarray(T, np.float32)
    data = np.asarray(data, np.float32)
    labels = np.asarray(labels, np.int32)

    import ml_dtypes
    bf16 = ml_dtypes.bfloat16

    ET = np.exp(T).astype(np.float32)
    ETs = (ET / CHAT).astype(np.float32)
    etf = np.zeros((128, 128), np.float32)
    etb = np.zeros((128, 128), np.float32)
    for c in range(4):
        etf[32 * c:32 * c + 32, 32 * c:32 * c + 32] = ETs
        etb[32 * c:32 * c + 32, 32 * c:32 * c + 32] = ETs.T
    oz = np.zeros((128, 4), np.float32)
    ob = np.zeros((4, 128), np.float32)
    on32 = np.zeros((128, K), np.float32)
    for c in range(4):
        oz[32 * c:32 * c + 32, c] = 1.0
        ob[c, 32 * c:32 * c + 32] = 1.0
        on32[32 * c:32 * c + 32, :] = np.eye(K, dtype=np.float32)
    id128 = np.eye(128, dtype=np.float32)
    wt4 = np.zeros((128, 4, 128), np.float32)
    for g in range(4):
        for c in range(4):
            wt4[:, g, 32 * c:32 * c + 32] = W.T[128 * g:128 * g + 128, :]

    nc = _CACHE.get("nc")
    if nc is None:
        nc = _build_module()
        _CACHE["nc"] = nc

    in_maps = []
    for core in range(NC):
        dcore = data[core * WPC:(core + 1) * WPC]        # [256, 64, 512]
        lcore = labels[core * WPC:(core + 1) * WPC]      # [256, 64]
        dc = dcore.reshape(4, 64, 64, D)                 # [c, wg, i, d]
        # dt[c, d, 64i+wg]
        dt = np.ascontiguousarray(dc.transpose(0, 3, 2, 1)).reshape(4, D, 4096)
        # dn[4096c + 64i + wg, d]
        dn = np.ascontiguousarray(dc.transpose(0, 2, 1, 3)).reshape(RPC, D)
        # oh[32c+k, 64i+wg]
        lc = lcore.reshape(4, 64, 64).transpose(0, 2, 1)  # [c, i, wg]
        oh = np.zeros((128, 4096), np.float32)
        ci, ii, wi = np.meshgrid(np.arange(4), np.arange(64), np.arange(64),
                                 indexing="ij")
        oh[32 * ci.ravel() + lc.ravel(), (64 * ii + wi).ravel()] = 1.0
        in_maps.append({
            "dt": dt.astype(bf16), "dn": dn.astype(bf16),
            "wt4": wt4.astype(bf16),
            "etf": etf, "etb": etb, "oz": oz, "ob": ob, "on32": on32,
            "id128": id128.astype(bf16), "oh": oh.astype(bf16),
        })

    _CACHE["last_in_maps"] = in_maps
    res = run_bass_kernel_spmd(nc, in_maps, list(range(NC)))
    results = res.results

    dw_sum = np.zeros((K, D), np.float64)
    Mmat = np.zeros((K, K), np.float64)
    for core in range(NC):
        r = results[core]
        dw_sum += r["dw"].astype(np.float64)
        ae = r["ae"].astype(np.float64)   # [128, 4096] packed, position-major
        be = r["be"].astype(np.float64)   # unnormalized
        # z per (c, wg) from AE[63]
        z = ae[:, 4032:4096].reshape(4, K, 64).sum(axis=1)   # [4, 64]
        rz = 1.0 / z
        # unpack to [c, i, wg, k]
        ae_n = ae.reshape(4, K, 64, 64).transpose(0, 2, 3, 1)
        be_n = be.reshape(4, K, 64, 64).transpose(0, 2, 3, 1)
        aer = ae_n[:, :M - 1]            # [c, 63, wg, k]
        ben = be_n[:, 1:]                # [c, 63, wg, k]
        Mmat += np.einsum("ciwk,ciwj,cw->kj", aer, ben, rz)

    counts = np.zeros((K, K), np.float64)
    np.add.at(counts, (labels[:, :-1].ravel(), labels[:, 1:].ravel()), 1.0)

    meandw = (-dw_sum / N).astype(np.float32)   # device computed (p1-oh)^T dn
    meandT = ((counts - (ET.astype(np.float64) / CHAT) * Mmat) / N
              ).astype(np.float32)
    return np.concatenate([meandw.ravel(), meandT.ravel()]).astype(np.float32)


# revision 9
# speedup vs baseline: 2.0427x; 1.5986x over previous
"""CRF layer gradient kernel for 8 TRN2 NeuronCores (v3).

Data-parallel over N=2048 words (256/core = 4 chains x 64 words).
Scaled forward-backward in exp domain, ETs = exp(T)/c:
  AE[i+1] = (AE[i] @ ETs) * E[i+1]   (AE[0] = E[0])
  BE[i-1] = (ETs @ BE[i]) * E[i-1]   (BE[63] = E[63], unnormalized)
fw/bw scans run CONCURRENTLY (z = sum_k AE[63] folded in afterwards via
ez = einv * (1/z) broadcast). Packing [128 = 4c x 32k, 4096 = 64 pos x 64 wg]
(POSITION-major: scan slices contiguous).

v3 perf structure:
- dt loaded as 8 x 2MB q-major units (host layout [8,128,8192]) split
  across both HWDGE rings (sync: q0-3, scalar: q7-4) -> big DMAs at line rate.
- dn loaded as 16 x 1MB partition-major units on the sync ring after dt.
- scan EMITTED INTERLEAVED with dots units so the PE FIFO never blocks the
  scan behind not-yet-needed dots matmuls; scan trails dt arrival.
- scan state in bf16 (fast matmuls), dots/dw matmuls col-tiled bf16.
- phase E per 128-column block: p1 -> G -> PE transpose -> col-tiled dw
  accumulation; dw reduced on device; ae/be out in bf16 on gpsimd ring.
"""

import sys

import numpy as np

sys.path.insert(0, "/opt/trn_rl_repo")

import concourse.bass as bass
import concourse.tile as tile
from concourse import bacc, mybir
from concourse.bass_utils import run_bass_kernel_spmd

N, M, K, D = 2048, 64, 32, 512
NC = 8
WPC = N // NC          # 256 words per core
RPC = WPC * M          # 16384 rows per core
CHAT = 60.0
F32 = mybir.dt.float32
BF16 = mybir.dt.bfloat16

_CACHE = {}


def _build_module():
    nc = bacc.Bacc("TRN2", target_bir_lowering=False, debug=False)

    dt_d = nc.dram_tensor("dt", [8, 128, 8192], BF16, kind="ExternalInput")
    dn_d = nc.dram_tensor("dn", [128, 65536], BF16, kind="ExternalInput")
    wt4_d = nc.dram_tensor("wt4", [128, 4, 128], BF16, kind="ExternalInput")
    etf_d = nc.dram_tensor("etf", [128, 128], BF16, kind="ExternalInput")
    etb_d = nc.dram_tensor("etb", [128, 128], BF16, kind="ExternalInput")
    oz_d = nc.dram_tensor("oz", [128, 4], BF16, kind="ExternalInput")
    ob_d = nc.dram_tensor("ob", [4, 128], F32, kind="ExternalInput")
    on32_d = nc.dram_tensor("on32", [128, K], F32, kind="ExternalInput")
    id128_d = nc.dram_tensor("id128", [128, 128], BF16, kind="ExternalInput")
    oh_d = nc.dram_tensor("oh", [128, 4096], BF16, kind="ExternalInput")
    dw_d = nc.dram_tensor("dw", [K, D], F32, kind="ExternalOutput")
    ae_d = nc.dram_tensor("ae", [128, 4096], BF16, kind="ExternalOutput")
    be_d = nc.dram_tensor("be", [128, 4096], BF16, kind="ExternalOutput")

    with tile.TileContext(nc) as tc:
        _kernel_body(tc, nc, dt_d, dn_d, wt4_d, etf_d, etb_d, oz_d, ob_d,
                     on32_d, id128_d, oh_d, dw_d, ae_d, be_d)
    nc.compile()
    return nc


def _kernel_body(tc, nc, dt_d, dn_d, wt4_d, etf_d, etb_d, oz_d, ob_d,
                 on32_d, id128_d, oh_d, dw_d, ae_d, be_d):
    from contextlib import ExitStack
    ctx = ExitStack()
    with ctx:
        consts = ctx.enter_context(tc.tile_pool(name="consts", bufs=1))
        big = ctx.enter_context(tc.tile_pool(name="big", bufs=1))
        dtp = ctx.enter_context(tc.tile_pool(name="dtp", bufs=4))
        dnp = ctx.enter_context(tc.tile_pool(name="dnp", bufs=8))
        scr = ctx.enter_context(tc.tile_pool(name="scr", bufs=6))
        gsbp = ctx.enter_context(tc.tile_pool(name="gsbp", bufs=3))

        # consts on the gpsimd (SWDGE) queue so they don't delay dt
        wt4_t = consts.tile([128, 4, 128], BF16)
        nc.gpsimd.dma_start(wt4_t[:], wt4_d.ap())
        etf_t = consts.tile([128, 128], BF16)
        nc.gpsimd.dma_start(etf_t[:], etf_d.ap())
        etb_t = consts.tile([128, 128], BF16)
        nc.gpsimd.dma_start(etb_t[:], etb_d.ap())
        oz_t = consts.tile([128, 4], BF16)
        nc.gpsimd.dma_start(oz_t[:], oz_d.ap())
        ob_t = consts.tile([4, 128], F32)
        nc.gpsimd.dma_start(ob_t[:], ob_d.ap())
        on32_t = consts.tile([128, K], F32)
        nc.gpsimd.dma_start(on32_t[:], on32_d.ap())
        id128_t = consts.tile([128, 128], BF16)
        nc.gpsimd.dma_start(id128_t[:], id128_d.ap())
        oh_t = big.tile([128, 4096], BF16, tag="oh")
        nc.gpsimd.dma_start(oh_t[:], oh_d.ap())

        e_t = big.tile([128, 4096], BF16, tag="e")
        ez_t = big.tile([128, 4096], BF16, tag="ez")   # einv, then einv*rz
        ae_t = big.tile([128, 4096], BF16, tag="ae")
        be_t = big.tile([128, 4096], BF16, tag="be")
        rzb_t = consts.tile([128, 64], F32)
        rz_t = consts.tile([4, 64], F32)

        # ---- input DMAs: dt on both rings, dn on sync after dt ----
        dt_tiles = {}
        for q in (0, 1, 2, 3):
            dt_tiles[q] = dtp.tile([128, 8192], BF16, tag="dt", name=f"dt{q}")
            nc.sync.dma_start(dt_tiles[q][:], dt_d.ap()[q])
        for q in (7, 6, 5, 4):
            dt_tiles[q] = dtp.tile([128, 8192], BF16, tag="dt", name=f"dt{q}")
            nc.scalar.dma_start(dt_tiles[q][:], dt_d.ap()[q])
        dn_tiles = {}
        for u in (0, 4, 8, 12, 1, 5, 9, 13, 2, 6, 10, 14, 3, 7, 11, 15):
            dn_tiles[u] = dnp.tile([128, 8, 512], BF16, tag="dn",
                                   name=f"dn{u}")
            nc.sync.dma_start(
                dn_tiles[u][:],
                dn_d.ap()[:, 4096 * u:4096 * u + 4096]
                .rearrange("p (j d) -> p j d", j=8))

        dot_psum = {}

        def emit_unit(dotp, q):
            """dots for positions 8q..8q+8: 16 col-tiled MMs + exp ACTs."""
            P = dotp.tile([128, 512], F32)
            dot_psum[q] = P
            for g in range(4):
                for c in range(4):
                    nc.tensor.matmul(
                        P[32 * c:32 * c + 32, :],
                        wt4_t[:, g, 32 * c:32 * c + 32],
                        dt_tiles[q][:, 512 * (4 * c + g):512 * (4 * c + g) + 512],
                        start=(g == 0), stop=(g == 3),
                        tile_position=(0, 32 * c))
            sl = slice(512 * q, 512 * q + 512)
            nc.scalar.activation(e_t[:, sl], P[:],
                                 mybir.ActivationFunctionType.Exp)
            nc.scalar.activation(ez_t[:, sl], P[:],
                                 mybir.ActivationFunctionType.Exp,
                                 scale=-1.0)

        # ---- Phase A+B interleaved: dots units woven into the scan ----
        with tc.tile_pool(name="dotp", bufs=3, space="PSUM") as dotp, \
             tc.tile_pool(name="scanp", bufs=3, space="PSUM") as scanp, \
             tc.tile_pool(name="zp", bufs=1, space="PSUM") as zp:
            emit_unit(dotp, 0)
            emit_unit(dotp, 7)

            nc.vector.tensor_copy(ae_t[:, 0:64], e_t[:, 0:64])
            nc.vector.tensor_copy(be_t[:, 4032:4096], e_t[:, 4032:4096])
            af = scanp.tile([128, 64], F32, tag="s")
            nc.tensor.matmul(af[:], etf_t[:], ae_t[:, 0:64],
                             start=True, stop=True)
            bb = scanp.tile([128, 64], F32, tag="s")
            nc.tensor.matmul(bb[:], etb_t[:], be_t[:, 4032:4096],
                             start=True, stop=True)

            unit_sched = {8: (1, 6), 16: (2, 5), 24: (3, 4)}
            for s in range(1, 64):
                if s in unit_sched:
                    for q in unit_sched[s]:
                        emit_unit(dotp, q)
                sf = slice(64 * s, 64 * s + 64)
                sb = slice(64 * (63 - s), 64 * (63 - s) + 64)
                nc.vector.tensor_mul(ae_t[:, sf], af[:], e_t[:, sf])
                nc.vector.tensor_mul(be_t[:, sb], bb[:], e_t[:, sb])
                if s < 63:
                    af = scanp.tile([128, 64], F32, tag="s")
                    nc.tensor.matmul(af[:], etf_t[:], ae_t[:, sf],
                                     start=True, stop=True)
                    bb = scanp.tile([128, 64], F32, tag="s")
                    nc.tensor.matmul(bb[:], etb_t[:], be_t[:, sb],
                                     start=True, stop=True)

            nc.gpsimd.dma_start(ae_d.ap(), ae_t[:])
            nc.gpsimd.dma_start(be_d.ap(), be_t[:])

            # ---- z and ez = einv * (1/z) broadcast ----
            z_ps = zp.tile([128, 64], F32, tag="z")
            nc.tensor.matmul(z_ps[0:4, :], oz_t[:], ae_t[:, 4032:4096],
                             start=True, stop=True)
            nc.vector.reciprocal(rz_t[:], z_ps[0:4, :])
            rzb_ps = zp.tile([128, 64], F32, tag="z")
            nc.tensor.matmul(rzb_ps[:], ob_t[:], rz_t[:],
                             start=True, stop=True)
            nc.vector.tensor_copy(rzb_t[:], rzb_ps[:])

        ez3 = ez_t[:].rearrange("p (i w) -> p i w", i=64)
        rz3 = rzb_t[:].unsqueeze(1)
        rz3b, ez3b = bass.broadcast_tensor_aps(rz3, ez3)
        nc.vector.tensor_mul(ez3, ez3b, rz3b)        # in-place einv -> ez

        # ---- Phase E: G = p1 - oh; PE transpose; col-tiled dw ----
        with tc.tile_pool(name="trp", bufs=2, space="PSUM") as trp, \
             tc.tile_pool(name="dwp", bufs=1, space="PSUM") as dwp, \
             tc.tile_pool(name="drp", bufs=1, space="PSUM") as drp:
            dwacc = dwp.tile([128, 512], F32)
            for jj in range(32):
                sl = slice(128 * jj, 128 * jj + 128)
                p1c = scr.tile([128, 128], F32, tag="p1")
                nc.vector.tensor_mul(p1c[:], ae_t[:, sl], be_t[:, sl])
                p1b = scr.tile([128, 128], BF16, tag="p1b")
                nc.vector.tensor_mul(p1b[:], p1c[:], ez_t[:, sl])
                gc = scr.tile([128, 128], BF16, tag="g")
                nc.vector.tensor_sub(gc[:], p1b[:], oh_t[:, sl])
                tr = trp.tile([128, 128], BF16)
                nc.tensor.transpose(tr[:], gc[:], id128_t[:])
                gsb = gsbp.tile([128, 128], BF16)
                nc.scalar.activation(gsb[:], tr[:],
                                     mybir.ActivationFunctionType.Copy)
                for c in range(4):
                    j = 32 * c + jj
                    u, slot = j // 8, j % 8
                    nc.tensor.matmul(dwacc[32 * c:32 * c + 32, :],
                                     gsb[:, 32 * c:32 * c + 32],
                                     dn_tiles[u][:, slot, :],
                                     start=(jj == 0), stop=(jj == 31),
                                     tile_position=(0, 32 * c))

            dwsb = gsbp.tile([128, 512], F32, tag="dwsb")
            nc.vector.tensor_copy(dwsb[:], dwacc[:])
            dwred = drp.tile([K, 512], F32)
            nc.tensor.matmul(dwred[:], on32_t[:], dwsb[:],
                             start=True, stop=True)
            dwout = gsbp.tile([K, 512], F32, tag="dwout")
            nc.vector.tensor_copy(dwout[:], dwred[:])
            nc.gpsimd.dma_start(dw_d.ap(), dwout[:])


def kernel(W, T, data, labels):
    W = np.asarray(W, np.float32)
    T = np.asarray(T, np.float32)
    data = np.asarray(data, np.float32)
    labels = np.asarray(labels, np.int32)

    import ml_dtypes
    bf16 = ml_dtypes.bfloat16

    ET = np.exp(T).astype(np.float32)
    ETs = (ET / CHAT).astype(np.float32)
    etf = np.zeros((128, 128), np.float32)
    etb = np.zeros((128, 128), np.float32)
    for c in range(4):
        etf[32 * c:32 * c + 32, 32 * c:32 * c + 32] = ETs
        etb[32 * c:32 * c + 32, 32 * c:32 * c + 32] = ETs.T
    oz = np.zeros((128, 4), np.float32)
    ob = np.zeros((4, 128), np.float32)
    on32 = np.zeros((128, K), np.float32)
    for c in range(4):
        oz[32 * c:32 * c + 32, c] = 1.0
        ob[c, 32 * c:32 * c + 32] = 1.0
        on32[32 * c:32 * c + 32, :] = np.eye(K, dtype=np.float32)
    id128 = np.eye(128, dtype=np.float32)
    wt4 = np.zeros((128, 4, 128), np.float32)
    for g in range(4):
        for c in range(4):
            wt4[:, g, 32 * c:32 * c + 32] = W.T[128 * g:128 * g + 128, :]

    nc = _CACHE.get("nc")
    if nc is None:
        nc = _build_module()
        _CACHE["nc"] = nc

    in_maps = []
    for core in range(NC):
        dcore = data[core * WPC:(core + 1) * WPC]        # [256, 64, 512]
        lcore = labels[core * WPC:(core + 1) * WPC]
        dc = dcore.reshape(4, 64, 64, D)                 # [c, wg, i, d]
        # dt4[q, p, (4c+g)*512 + f'] = data[c, wg, 8q+i', 128g+p]
        #   where f' = 64*i' + wg, i = 8q + i'
        # dc -> [c, g, p, q, i', wg]
        dtt = dc.transpose(0, 3, 2, 1).reshape(4, 4, 128, 8, 8, 64)
        # dims now [c, g, p, q, i', wg] -> want [q, p, c, g, i', wg]
        dt4 = np.ascontiguousarray(dtt.transpose(3, 2, 0, 1, 4, 5)
                                   ).reshape(8, 128, 8192)
        # dn2[p, j*512 + d] = data row (128j + p) in position-major order
        dnn = dc.transpose(0, 2, 1, 3).reshape(RPC, D)   # [4096c+64i+wg, d]
        dn2 = np.ascontiguousarray(
            dnn.reshape(128, 128, D).transpose(1, 0, 2)).reshape(128, 65536)
        lc = lcore.reshape(4, 64, 64).transpose(0, 2, 1)  # [c, i, wg]
        oh = np.zeros((128, 4096), np.float32)
        ci, ii, wi = np.meshgrid(np.arange(4), np.arange(64), np.arange(64),
                                 indexing="ij")
        oh[32 * ci.ravel() + lc.ravel(), (64 * ii + wi).ravel()] = 1.0
        in_maps.append({
            "dt": dt4.astype(bf16), "dn": dn2.astype(bf16),
            "wt4": wt4.astype(bf16),
            "etf": etf.astype(bf16), "etb": etb.astype(bf16),
            "oz": oz.astype(bf16), "ob": ob, "on32": on32,
            "id128": id128.astype(bf16), "oh": oh.astype(bf16),
        })

    _CACHE["last_in_maps"] = in_maps
    res = run_bass_kernel_spmd(nc, in_maps, list(range(NC)))
    results = res.results

    dw_sum = np.zeros((K, D), np.float64)
    Mmat = np.zeros((K, K), np.float64)
    for core in range(NC):
        r = results[core]
        dw_sum += r["dw"].astype(np.float64)
        ae = r["ae"].astype(np.float64)   # [128, 4096] packed bf16
        be = r["be"].astype(np.float64)
        z = ae[:, 4032:4096].reshape(4, K, 64).sum(axis=1)   # [4, 64]
        rz = 1.0 / z
        ae_n = ae.reshape(4, K, 64, 64).transpose(0, 2, 3, 1)  # [c,i,wg,k]
        be_n = be.reshape(4, K, 64, 64).transpose(0, 2, 3, 1)
        Mmat += np.einsum("ciwk,ciwj,cw->kj",
                          ae_n[:, :M - 1], be_n[:, 1:], rz)

    counts = np.zeros((K, K), np.float64)
    np.add.at(counts, (labels[:, :-1].ravel(), labels[:, 1:].ravel()), 1.0)

    meandw = (-dw_sum / N).astype(np.float32)
    meandT = ((counts - (ET.astype(np.float64) / CHAT) * Mmat) / N
              ).astype(np.float32)
    return np.concatenate([meandw.ravel(), meandT.ravel()]).astype(np.float32)
